# revision 17
# baseline (speedup 1.0000x reference)
"""Bass/Trainium2 kernel for nn_Differential_Attention_60825326846200.

Mathematical reduction of the reference:
  scores[b,h,i,j] = (sum_d q[b,h,i,d] - k[b,h,i,d]) / sqrt(DH) + mask[b,i]
is constant over the key index j, so the softmax over j is exactly the
uniform distribution (1/S) regardless of q, k, and the mask.  Hence
  ctx[b,h,i,:] = mean_j v[b,h,j,:]          (independent of i)
  out[b,i,:]   = (mean_j hidden_b[b,j,:]) @ Wv.T + bv   for every i.
The q/k projections and the attention mask cancel exactly, and the output
is rank-1 along the sequence axis: 2048 identical rows per batch.

ONE SPMD launch, contraction-sharded (no cross-core exchange needed):
core c owns HID columns d in [128c, 128c+128).

  Because the hidden dim (not the sequence) is sharded, each core's
  sequence reduction is COMPLETE for its slice: it reads
  hidden_b[:, :, d_c], reduces over all 2048 positions on the DVE
  (partitions = the 128 hidden columns, so m[d, b] lands already
  transposed for the projection lhsT), then contracts its 128 columns
  with its Wv slice -> z_c[b, o], a contraction-partial of the unique
  output row.  Core 0's bias input carries bv (others zeros), added via
  a rank-1 PE matmul into the same PSUM accumulation; m is pre-scaled
  by the exact 1/S so the PSUM holds final values and the evacuation is
  a plain copy.

  Host unshard = the standard gather for contraction sharding: sum the
  8 partials [2, 1024] and broadcast over the sequence axis (the output
  is rank-1: every row within a batch is the same vector).

  The inputs stream in bf16 (cast on the host while laying out the
  shards -- the 2e-2 tolerance is far above the ~3.6e-3 this lands at,
  and the f32 baseline already ran its matmuls in TF32-width float32r):
  1.26MB in / 4KB out per core, balanced 640KB per HWDGE ring with each
  wvt half on the ring that feeds its projection half.  GpSimd
  tree-folds the last piece (1024 -> 256) so the DVE's final reduce is
  short.  Measured HW behavior baked into the design: HWDGE reads are
  ~95GB/s per ring on 2KB runs / ~130GB/s on 4KB runs, and the two
  rings share the 16 SDMA engines; SWDGE (gpsimd dma) is ~30-50GB/s --
  good only for tiny loads (and the only path supporting dma accum);
  partition-sliced DMAs use only half the SDMA engines (never split
  below 128 partitions); free-axis tensor_reduce is DVE-only at
  ~1.1ns/element regardless of dtype (+~250ns per op); the first ACT op
  would emit a ~1.3us ACT_TABLE_LOAD ahead of the scalar ring's DMA
  issues, so no ACT ops are used at all; f32r matmuls need a moving
  free dim >= 2.  Fixed costs dominate what remains: ~7us prolog
  (pre-body semaphore waits + instruction load) and ~2.9us of counted
  epilog per launch, which is why everything fits in ONE launch.
"""

import numpy as np
import ml_dtypes

import concourse.bacc as bacc
import concourse.mybir as mybir
import concourse.tile as tile
from concourse.bass_utils import run_bass_kernel_spmd

N_CORES = 8
B, S, HID = 2, 2048, 1024
D_LOC = HID // N_CORES  # 128 hidden columns owned per core
SH = S // 2  # seq halves: 4 stream pieces (batch x half), reduced as they land
F32 = mybir.dt.float32
F32R = mybir.dt.float32r
BF16 = mybir.dt.bfloat16
NPBF16 = ml_dtypes.bfloat16

_compiled = None


def _new_nc():
    return bacc.Bacc(
        "TRN2",
        target_bir_lowering=False,
        debug=False,
        enable_asserts=False,
        num_devices=N_CORES,
    )


def _build():
    """Single launch: complete seq-reduction of this core's column slice,
    projection through its Wv rows, contraction-partial out.
    Inputs:
      "hbt" [128, sum(2*scs)] bf16, chunk-major flat cols (k, b, s):
        hbt[d, (k, b, s)] = hb[b, chunk_k_start + s, 128*core + d]
      "wvt" [128, HID] bf16: wvt[d, o] = Wv[o, 128*core+d]
      "bvS" [1, HID] bf16: bv on core 0, zeros elsewhere
      "consts" [1, 2] bf16: ones (bias-matmul stationary)
    Output "zout" [B, HID] f32: this core's contraction-partial of the
    unique output row (bias included on core 0, 1/S applied)."""
    nc = _new_nc()
    hbt = nc.dram_tensor("hbt", [128, B, 2, SH], BF16, kind="ExternalInput").ap()
    wvt = nc.dram_tensor("wvt", [128, HID], BF16, kind="ExternalInput").ap()
    # bvS[0, 0:2] = ones (bias-matmul stationary), bvS[0, 2:] = bv or 0
    bvS = nc.dram_tensor("bvS", [1, 2 + HID], BF16, kind="ExternalInput").ap()
    zout = nc.dram_tensor("zout", [B, HID], BF16, kind="ExternalOutput").ap()

    with tile.TileContext(nc) as tc:
        with (
            tc.tile_pool(name="big", bufs=1) as big,
            tc.tile_pool(name="small", bufs=1) as small,
            tc.tile_pool(name="psum", bufs=1, space="PSUM") as psum,
        ):
            hb_sb = big.tile([128, B, 2, SH], BF16)
            wvt_sb = big.tile([128, HID], BF16)
            bvS_sb = small.tile([1, 2 + HID], BF16)

            # Four 256KB pieces (batch x seq-half), two per HWDGE ring,
            # reduced by the DVE in arrival order.  The tiny bias row
            # rides SWDGE (off the HWDGE rings); wvt goes last on the
            # sync ring -- it gates only the final projection, which also
            # needs m (ready ~1.5us after the last hbt piece).
            nc.gpsimd.dma_start(bvS_sb[:], bvS[:])
            nc.sync.dma_start(hb_sb[:, 0, 0], hbt[:, 0, 0])
            nc.scalar.dma_start(hb_sb[:, 1, 0], hbt[:, 1, 0])
            nc.sync.dma_start(hb_sb[:, 0, 1], hbt[:, 0, 1])
            nc.scalar.dma_start(hb_sb[:, 1, 1], hbt[:, 1, 1])
            # wvt halves ride one ring each (balances the rings at 640KB;
            # each half feeds its own projection half)
            nc.sync.dma_start(wvt_sb[:, 0:512], wvt[:, 0:512])
            nc.scalar.dma_start(wvt_sb[:, 512:1024], wvt[:, 512:1024])

            ones2 = bvS_sb[0:1, 0:2]
            scratch = small.tile([128, 512], BF16)
            nc.vector.memset(scratch[:], 1.0)
            # PE and DVE clock-gate warmups while the first DMAs stream
            pw = psum.tile([2, 512], F32, name="pwarm", tag="pwarm")
            for _ in range(6):
                nc.tensor.matmul(
                    pw[:], lhsT=scratch[:, 0:2], rhs=scratch[:], start=True, stop=True
                )
            # seq reduction: one [128, SH] free-axis DVE reduce per
            # (batch, half) in arrival order, a tiny combine, then the
            # exact 1/S mean scaling into the projection's bf16 stationary
            # GpSimd (idle after its one DMA issue) tree-folds the last
            # piece 1024 -> 512 -> 256 so the DVE's final reduce is 4x
            # shorter; everything else reduces on the DVE in arrival order
            fold1 = small.tile([128, 512], BF16)
            fold2 = small.tile([128, 256], BF16)
            with nc.allow_low_precision(reason="bf16 partial-sum folds"):
                nc.gpsimd.tensor_add(
                    fold1[:], hb_sb[:, 1, 1, 0:512], hb_sb[:, 1, 1, 512:1024]
                )
                nc.gpsimd.tensor_add(fold2[:], fold1[:, 0:256], fold1[:, 256:512])
            mtmp = small.tile([128, B * 2], BF16)
            mcmb = small.tile([128, B], BF16)
            m_sb = small.tile([128, B], BF16)
            with nc.allow_low_precision(reason="bf16 in, f32 accumulate"):
                for b2, half in ((0, 0), (1, 0), (0, 1)):
                    nc.vector.reduce_sum(
                        mtmp[:, b2 * 2 + half : b2 * 2 + half + 1],
                        hb_sb[:, b2, half],
                        axis=mybir.AxisListType.X,
                    )
                nc.vector.reduce_sum(
                    mtmp[:, 3:4], fold2[:], axis=mybir.AxisListType.X
                )
                nc.vector.reduce_sum(
                    mcmb[:],
                    mtmp[:].rearrange("p (b h) -> p b h", b=B),
                    axis=mybir.AxisListType.X,
                )
                nc.vector.tensor_scalar_mul(m_sb[:], mcmb[:], 1.0 / S)
            # keep the PE clock ungated between the warmups and the
            # projection, fed by batch-0's first piece
            for _ in range(2):
                nc.tensor.matmul(
                    pw[:],
                    lhsT=hb_sb[:, 0, 0, 0:2],
                    rhs=hb_sb[:, 0, 0, 0:512],
                    start=True,
                    stop=True,
                )

            # projection: z[b, o] = sum_d m[d, b]*wvt[d, o] + bv[o]; the
            # rank-1 bias matmul starts each accumulation group (it
            # depends only on the tiny early loads)
            z_sb = small.tile([B, HID], BF16)
            for h, weng in ((0, nc.sync), (1, nc.scalar)):
                zp = psum.tile([B, 512], F32, name=f"z{h}", tag=f"z{h}")
                nc.tensor.matmul(
                    zp[:],
                    lhsT=ones2,
                    rhs=bvS_sb[:, 2 + h * 512 : 2 + (h + 1) * 512],
                    start=True,
                    stop=False,
                )
                nc.tensor.matmul(
                    zp[:],
                    lhsT=m_sb[:],
                    rhs=wvt_sb[:, h * 512 : (h + 1) * 512],
                    start=False,
                    stop=True,
                )
                # plain evac (m carried the 1/S, bias is in the PSUM);
                # each half's write leaves on its own HWDGE ring as soon
                # as its evac lands
                with nc.allow_low_precision(reason="bf16 partials, host sums f32"):
                    nc.vector.tensor_copy(z_sb[:, h * 512 : (h + 1) * 512], zp[:])
                weng.dma_start(
                    zout[:, h * 512 : (h + 1) * 512], z_sb[:, h * 512 : (h + 1) * 512]
                )
    nc.compile()
    return nc


def get_nc():
    global _compiled
    if _compiled is None:
        _compiled = _build()
    return _compiled


def make_in_maps(inputs):
    hb = np.asarray(inputs["hidden_states_b"], dtype=np.float32)
    Wv = np.asarray(inputs["Wv"], dtype=np.float32)
    bv = np.asarray(inputs["bv"], dtype=np.float32)
    bvS = np.zeros((N_CORES, 1, 2 + HID), dtype=NPBF16)
    bvS[:, 0, 0:2] = 1  # bias-matmul stationary ones
    bvS[0, 0, 2:] = bv.astype(NPBF16)
    maps = []
    for c in range(N_CORES):
        sl = hb[:, :, c * D_LOC : (c + 1) * D_LOC].astype(NPBF16)  # [B, S, 128]
        # hbt[d, b, half, s] = hb[b, half*SH + s, c*128 + d]
        t = sl.reshape(B, 2, SH, D_LOC).transpose(3, 0, 1, 2)
        wt = Wv[:, c * D_LOC : (c + 1) * D_LOC].T.astype(NPBF16)  # [128 d, HID o]
        maps.append(
            {
                "hbt": np.ascontiguousarray(t),
                "wvt": np.ascontiguousarray(wt),
                "bvS": bvS[c],
            }
        )
    return maps


def combine(results):
    # unshard for contraction sharding: sum the 8 partials (bias was
    # folded into core 0's partial, 1/S scaling done on-device), then
    # broadcast the unique per-batch row over the sequence axis
    z = results[0]["zout"].astype(np.float32)
    for c in range(1, N_CORES):
        z += results[c]["zout"].astype(np.float32)
    return np.ascontiguousarray(np.broadcast_to(z[:, None, :], (B, S, HID)))


def kernel(**inputs) -> np.ndarray:
    nc = get_nc()
    res = run_bass_kernel_spmd(nc, make_in_maps(inputs), list(range(N_CORES)))
    return combine(res.results)


# revision 18
# speedup vs baseline: 1.0250x; 1.0250x over previous
"""Bass/Trainium2 kernel for nn_Differential_Attention_60825326846200.

Mathematical reduction of the reference:
  scores[b,h,i,j] = (sum_d q[b,h,i,d] - k[b,h,i,d]) / sqrt(DH) + mask[b,i]
is constant over the key index j, so the softmax over j is exactly the
uniform distribution (1/S) regardless of q, k, and the mask.  Hence
  ctx[b,h,i,:] = mean_j v[b,h,j,:]          (independent of i)
  out[b,i,:]   = (mean_j hidden_b[b,j,:]) @ Wv.T + bv   for every i.
The q/k projections and the attention mask cancel exactly, and the output
is rank-1 along the sequence axis: 2048 identical rows per batch.

ONE SPMD launch, contraction-sharded (no cross-core exchange needed):
core c owns HID columns d in [128c, 128c+128).

  Because the hidden dim (not the sequence) is sharded, each core's
  sequence reduction is COMPLETE for its slice: it reads
  hidden_b[:, :, d_c], reduces over all 2048 positions (partitions = the
  128 hidden columns, so m[d, b] lands already transposed for the
  projection lhsT), then contracts its 128 columns with its Wv slice ->
  z_c[b, o], a contraction-partial of the unique output row.  Core 0's
  bias input carries bv (others zeros), added via a rank-1 PE matmul
  into the same PSUM accumulation; m is pre-scaled by the exact 1/S so
  the PSUM holds final values and the evacuation is a plain copy.

  Host unshard = the standard gather for contraction sharding: sum the
  8 partials [2, 1024] and broadcast over the sequence axis (the output
  is rank-1: every row within a batch is the same vector).

  The inputs stream in bf16 (cast on the host while laying out the
  shards -- the 2e-2 tolerance is far above the ~3.6e-3 this lands at,
  and the f32 baseline already ran its matmuls in TF32-width float32r):
  1.26MB in / 4KB out per core.  Measured HW behavior baked in:
  - HWDGE rings cost ~21ns per descriptor with a ~131GB/s byte ceiling;
    a descriptor is one per-partition contiguous run, so the stream is
    THREE transfers only (each batch row = one 4KB run per partition,
    wvt whole = 2KB runs).  Column-splitting wvt (1KB runs) measured
    descriptor-limited at ~47GB/s.  SWDGE (gpsimd dma) is ~30-50GB/s,
    tiny loads only; partition-sliced DMAs use only half the SDMA
    engines.
  - The sequence reduction runs on TWO engines in parallel: DVE
    free-axis reduce_sum for batch 0, ACT activation(Copy, accum_out =
    per-partition running sum) for batch 1.  Both are ~1ns/element;
    neither alone can hide under the stream tail.  The ~1.3us
    ACT_TABLE_LOAD runs on the ACT engine concurrently with the scalar
    sequencer's DMA issues (it does NOT block the ring); a dummy early
    ACT op pins it to body start.
  - PSUM evac: z half 0 through ACT, half 1 through the DVE, in
    parallel; each half's 1KB-run write leaves on its own HWDGE ring.
  - f32r matmuls need a moving free dim >= 2; PSUM banks are 2KB per
    partition so z is two [2, 512] tiles.
  - Fixed costs dominate what remains: ~7us prolog (pre-body semaphore
    waits + instruction load) and ~2.9us of counted epilog per launch,
    which is why everything fits in ONE launch.
"""

import numpy as np
import ml_dtypes

import concourse.bacc as bacc
import concourse.mybir as mybir
import concourse.tile as tile
from concourse.bass_utils import run_bass_kernel_spmd

N_CORES = 8
B, S, HID = 2, 2048, 1024
D_LOC = HID // N_CORES  # 128 hidden columns owned per core
F32 = mybir.dt.float32
F32R = mybir.dt.float32r
BF16 = mybir.dt.bfloat16
NPBF16 = ml_dtypes.bfloat16

_compiled = None


def _new_nc():
    return bacc.Bacc(
        "TRN2",
        target_bir_lowering=False,
        debug=False,
        enable_asserts=False,
        num_devices=N_CORES,
    )


def _build():
    """Single launch: complete seq-reduction of this core's column slice,
    projection through its Wv rows, contraction-partial out.
    Inputs:
      "hbt" [128, B, S] bf16: hbt[d, b, s] = hb[b, s, 128*core + d]
      "wvt" [128, HID] bf16: wvt[d, o] = Wv[o, 128*core+d]
      "bvS" [1, 2+HID] bf16: cols 0:2 ones (bias-matmul stationary),
        cols 2: bv on core 0 / zeros elsewhere
    Output "zout" [B, HID] bf16: this core's contraction-partial of the
    unique output row (bias included on core 0, 1/S applied)."""
    nc = _new_nc()
    hbt = nc.dram_tensor("hbt", [128, B, S], BF16, kind="ExternalInput").ap()
    wvt = nc.dram_tensor("wvt", [128, HID], BF16, kind="ExternalInput").ap()
    bvS = nc.dram_tensor("bvS", [1, 2 + HID], BF16, kind="ExternalInput").ap()
    zout = nc.dram_tensor("zout", [B, HID], BF16, kind="ExternalOutput").ap()

    with tile.TileContext(nc) as tc:
        with (
            tc.tile_pool(name="big", bufs=1) as big,
            tc.tile_pool(name="small", bufs=1) as small,
            tc.tile_pool(name="psum", bufs=1, space="PSUM") as psum,
        ):
            hb_sb = big.tile([128, B, S], BF16)
            wvt_sb = big.tile([128, HID], BF16)
            bvS_sb = small.tile([1, 2 + HID], BF16)

            # three big transfers (descriptor-minimal); wvt last on the
            # sync ring -- it gates only the final projection, which also
            # needs m; tiny bias row on SWDGE off the HWDGE rings
            nc.gpsimd.dma_start(bvS_sb[:], bvS[:])
            nc.sync.dma_start(hb_sb[:, 0], hbt[:, 0])
            nc.scalar.dma_start(hb_sb[:, 1], hbt[:, 1])
            nc.sync.dma_start(wvt_sb[:], wvt[:])

            ones2 = bvS_sb[0:1, 0:2]
            scratch = small.tile([128, 512], BF16)
            nc.vector.memset(scratch[:], 1.0)
            # PE clock-gate warmups while the DMAs stream; the dummy ACT
            # op pins the ~1.3us ACT_TABLE_LOAD to body start (it runs on
            # the ACT engine, concurrent with the scalar sequencer's DMA
            # issues -- it does not block the ring)
            pw = psum.tile([2, 512], F32, name="pwarm", tag="pwarm")
            for _ in range(6):
                nc.tensor.matmul(
                    pw[:], lhsT=scratch[:, 0:2], rhs=scratch[:], start=True, stop=True
                )
            act_warm = small.tile([1, 2], BF16)
            with nc.allow_low_precision(reason="dummy table-load trigger"):
                nc.scalar.copy(act_warm[:], scratch[0:1, 0:2])

            # seq reduction, one batch per engine in parallel: DVE
            # free-axis reduce for batch 0; ACT activation(Copy) with
            # accum_out (per-partition sum over the free axis) for batch
            # 1, its full-size out going to a scratch tile
            act_sink = big.tile([128, S], BF16)
            mtmp = small.tile([128, B], F32)
            m_sb = small.tile([128, B], BF16)
            with nc.allow_low_precision(reason="bf16 in, f32 accumulate"):
                nc.vector.reduce_sum(
                    mtmp[:, 0:1], hb_sb[:, 0], axis=mybir.AxisListType.X
                )
                nc.scalar.activation(
                    act_sink[:],
                    hb_sb[:, 1],
                    mybir.ActivationFunctionType.Copy,
                    accum_out=mtmp[:, 1:2],
                )
                # exact 1/S mean scaling into the projection's stationary
                nc.vector.tensor_scalar_mul(m_sb[:], mtmp[:], 1.0 / S)
            # keep the PE clock ungated between the warmups and the
            # projection, fed by the freshly-landed streams
            for _ in range(2):
                nc.tensor.matmul(
                    pw[:],
                    lhsT=hb_sb[:, 0, 0:2],
                    rhs=hb_sb[:, 0, 0:512],
                    start=True,
                    stop=True,
                )
            for _ in range(2):
                nc.tensor.matmul(
                    pw[:],
                    lhsT=wvt_sb[:, 0:2],
                    rhs=wvt_sb[:, 0:512],
                    start=True,
                    stop=True,
                )

            # projection: z[b, o] = sum_d m[d, b]*wvt[d, o] + bv[o]; the
            # rank-1 bias matmul starts each accumulation group (it
            # depends only on the tiny early loads)
            z_sb = small.tile([B, HID], BF16)
            for h, weng in ((0, nc.scalar), (1, nc.sync)):
                zp = psum.tile([B, 512], F32, name=f"z{h}", tag=f"z{h}")
                nc.tensor.matmul(
                    zp[:],
                    lhsT=ones2,
                    rhs=bvS_sb[:, 2 + h * 512 : 2 + (h + 1) * 512],
                    start=True,
                    stop=False,
                )
                nc.tensor.matmul(
                    zp[:],
                    lhsT=m_sb[:],
                    rhs=wvt_sb[:, h * 512 : (h + 1) * 512],
                    start=False,
                    stop=True,
                )
                # evac (m carried the 1/S, bias is in the PSUM): z0
                # through ACT, z1 through the DVE, in parallel; each
                # half's write leaves on its own HWDGE ring right after
                with nc.allow_low_precision(reason="bf16 partials, host sums f32"):
                    if h == 0:
                        nc.scalar.copy(z_sb[:, 0:512], zp[:])
                    else:
                        nc.vector.tensor_copy(z_sb[:, 512:1024], zp[:])
                weng.dma_start(
                    zout[:, h * 512 : (h + 1) * 512], z_sb[:, h * 512 : (h + 1) * 512]
                )
    nc.compile()
    return nc


def get_nc():
    global _compiled
    if _compiled is None:
        _compiled = _build()
    return _compiled


def make_in_maps(inputs):
    hb = np.asarray(inputs["hidden_states_b"], dtype=np.float32)
    Wv = np.asarray(inputs["Wv"], dtype=np.float32)
    bv = np.asarray(inputs["bv"], dtype=np.float32)
    bvS = np.zeros((N_CORES, 1, 2 + HID), dtype=NPBF16)
    bvS[:, 0, 0:2] = 1  # bias-matmul stationary ones
    bvS[0, 0, 2:] = bv.astype(NPBF16)
    maps = []
    for c in range(N_CORES):
        sl = hb[:, :, c * D_LOC : (c + 1) * D_LOC].astype(NPBF16)  # [B, S, 128]
        # hbt[d, b, s] = hb[b, s, c*128 + d]: one 4KB run per (d, b)
        t = sl.transpose(2, 0, 1)
        wt = Wv[:, c * D_LOC : (c + 1) * D_LOC].T.astype(NPBF16)  # [128 d, HID o]
        maps.append(
            {
                "hbt": np.ascontiguousarray(t),
                "wvt": np.ascontiguousarray(wt),
                "bvS": bvS[c],
            }
        )
    return maps


def combine(results):
    # unshard for contraction sharding: sum the 8 partials (bias was
    # folded into core 0's partial, 1/S scaling done on-device), then
    # broadcast the unique per-batch row over the sequence axis
    z = results[0]["zout"].astype(np.float32)
    for c in range(1, N_CORES):
        z += results[c]["zout"].astype(np.float32)
    return np.ascontiguousarray(np.broadcast_to(z[:, None, :], (B, S, HID)))


def kernel(**inputs) -> np.ndarray:
    nc = get_nc()
    res = run_bass_kernel_spmd(nc, make_in_maps(inputs), list(range(N_CORES)))
    return combine(res.results)


# revision 19
# speedup vs baseline: 1.0664x; 1.0404x over previous
"""Bass/Trainium2 kernel for nn_Differential_Attention_60825326846200.

Mathematical reduction of the reference:
  scores[b,h,i,j] = (sum_d q[b,h,i,d] - k[b,h,i,d]) / sqrt(DH) + mask[b,i]
is constant over the key index j, so the softmax over j is exactly the
uniform distribution (1/S) regardless of q, k, and the mask.  Hence
  ctx[b,h,i,:] = mean_j v[b,h,j,:]          (independent of i)
  out[b,i,:]   = (mean_j hidden_b[b,j,:]) @ Wv.T + bv   for every i.
The q/k projections and the attention mask cancel exactly, and the output
is rank-1 along the sequence axis: 2048 identical rows per batch.

ONE SPMD launch, contraction-sharded (no cross-core exchange needed):
core c owns HID columns d in [128c, 128c+128).

  Because the hidden dim (not the sequence) is sharded, each core's
  sequence reduction is COMPLETE for its slice: it reads
  hidden_b[:, :, d_c], reduces over all 2048 positions (partitions = the
  128 hidden columns, so m[d, b] lands already transposed for the
  projection lhsT), then contracts its 128 columns with its Wv slice ->
  z_c[b, o], a contraction-partial of the unique output row.  Core 0's
  bias input carries bv (others zeros), added via a rank-1 PE matmul
  into the same PSUM accumulation; m is pre-scaled by the exact 1/S so
  the PSUM holds final values and the evacuation is a plain copy.

  Host unshard = the standard gather for contraction sharding: sum the
  8 partials [2, 1024] and broadcast over the sequence axis (the output
  is rank-1: every row within a batch is the same vector).

  The inputs stream in bf16 (cast on the host while laying out the
  shards -- the 2e-2 tolerance is far above the ~3.6e-3 this lands at,
  and the f32 baseline already ran its matmuls in TF32-width float32r):
  1.26MB in / 4KB out per core.  Measured HW behavior baked in:
  - HWDGE rings cost ~21ns per descriptor with a ~131GB/s byte ceiling;
    a descriptor is one per-partition contiguous run, so the stream is
    THREE transfers only (each batch row = one 4KB run per partition,
    wvt whole = 2KB runs).  Column-splitting wvt (1KB runs) measured
    descriptor-limited at ~47GB/s.  SWDGE (gpsimd dma) is ~30-50GB/s,
    tiny loads only; partition-sliced DMAs use only half the SDMA
    engines.
  - The sequence reduction runs on TWO engines in parallel: DVE
    free-axis reduce_sum for batch 0, ACT activation(Copy, accum_out =
    per-partition running sum) for batch 1.  Both are ~1ns/element;
    neither alone can hide under the stream tail.  The ~1.3us
    ACT_TABLE_LOAD runs on the ACT engine concurrently with the scalar
    sequencer's DMA issues (it does NOT block the ring); a dummy early
    ACT op pins it to body start.
  - PSUM evac: z half 0 through ACT, half 1 through the DVE, in
    parallel; each half's 1KB-run write leaves on its own HWDGE ring.
  - f32r matmuls need a moving free dim >= 2; PSUM banks are 2KB per
    partition so z is two [2, 512] tiles.
  - Fixed costs dominate what remains: ~7us prolog (pre-body semaphore
    waits + instruction load) and ~2.9us of counted epilog per launch,
    which is why everything fits in ONE launch.
"""

import numpy as np
import ml_dtypes

import concourse.bacc as bacc
import concourse.mybir as mybir
import concourse.tile as tile
from concourse.bass_utils import run_bass_kernel_spmd

N_CORES = 8
B, S, HID = 2, 2048, 1024
D_LOC = HID // N_CORES  # 128 hidden columns owned per core
F32 = mybir.dt.float32
F32R = mybir.dt.float32r
BF16 = mybir.dt.bfloat16
NPBF16 = ml_dtypes.bfloat16

_compiled = None


def _new_nc():
    return bacc.Bacc(
        "TRN2",
        target_bir_lowering=False,
        debug=False,
        enable_asserts=False,
        num_devices=N_CORES,
    )


def _build():
    """Single launch: complete seq-reduction of this core's column slice,
    projection through its Wv rows, contraction-partial out.
    Inputs:
      "hbt" [128, B, S] bf16: hbt[d, b, s] = hb[b, s, 128*core + d]
      "wvt" [128, HID] bf16: wvt[d, o] = Wv[o, 128*core+d]
      "bvS" [1, 2+HID] bf16: cols 0:2 ones (bias-matmul stationary),
        cols 2: bv on core 0 / zeros elsewhere
    Output "zout" [B, HID] bf16: this core's contraction-partial of the
    unique output row (bias included on core 0, 1/S applied)."""
    nc = _new_nc()
    hbt = nc.dram_tensor("hbt", [128, B, S], BF16, kind="ExternalInput").ap()
    wvt = nc.dram_tensor("wvt", [128, HID], BF16, kind="ExternalInput").ap()
    bvS = nc.dram_tensor("bvS", [1, 2 + HID], BF16, kind="ExternalInput").ap()
    zout = nc.dram_tensor("zout", [B, HID], BF16, kind="ExternalOutput").ap()

    with tile.TileContext(nc) as tc:
        with (
            tc.tile_pool(name="big", bufs=1) as big,
            tc.tile_pool(name="small", bufs=1) as small,
            tc.tile_pool(name="psum", bufs=1, space="PSUM") as psum,
        ):
            hb_sb = big.tile([128, B, S], BF16)
            wvt_sb = big.tile([128, HID], BF16)
            bvS_sb = small.tile([1, 2 + HID], BF16)

            # three big transfers (descriptor-minimal); wvt last on the
            # sync ring -- it gates only the final projection, which also
            # needs m; tiny bias row on SWDGE off the HWDGE rings
            nc.gpsimd.dma_start(bvS_sb[:], bvS[:])
            nc.sync.dma_start(hb_sb[:, 0], hbt[:, 0])
            nc.scalar.dma_start(hb_sb[:, 1], hbt[:, 1])
            nc.sync.dma_start(wvt_sb[:], wvt[:])

            ones2 = bvS_sb[0:1, 0:2]
            scratch = small.tile([128, 512], BF16)
            nc.vector.memset(scratch[:], 1.0)
            # PE clock-gate warmups while the DMAs stream; the dummy ACT
            # op pins the ~1.3us ACT_TABLE_LOAD to body start (it runs on
            # the ACT engine, concurrent with the scalar sequencer's DMA
            # issues -- it does not block the ring)
            pw = psum.tile([2, 512], F32, name="pwarm", tag="pwarm")
            for _ in range(6):
                nc.tensor.matmul(
                    pw[:], lhsT=scratch[:, 0:2], rhs=scratch[:], start=True, stop=True
                )
            act_warm = small.tile([1, 2], BF16)
            with nc.allow_low_precision(reason="dummy table-load trigger"):
                nc.scalar.copy(act_warm[:], scratch[0:1, 0:2])

            # seq reduction, one batch per engine in parallel: DVE
            # free-axis reduce for batch 0; ACT activation(Copy) with
            # accum_out (per-partition sum over the free axis) for batch
            # 1, its full-size out going to a scratch tile
            act_sink = big.tile([128, S], BF16)
            mtmp = small.tile([128, 1], F32)
            m_sb = small.tile([128, B], BF16)
            with nc.allow_low_precision(reason="bf16 in, f32 accumulate"):
                nc.vector.reduce_sum(mtmp[:], hb_sb[:, 0], axis=mybir.AxisListType.X)
                # exact 1/S mean scaling folded into each engine's own
                # column of the projection stationary (no cross-engine
                # combine step)
                nc.vector.tensor_scalar_mul(m_sb[:, 0:1], mtmp[:], 1.0 / S)
                nc.scalar.activation(
                    act_sink[:],
                    hb_sb[:, 1],
                    mybir.ActivationFunctionType.Copy,
                    scale=1.0 / S,
                    accum_out=m_sb[:, 1:2],
                )
            # keep the PE clock ungated between the warmups and the
            # projection (kept off the critical path: warms fed by the
            # first stream only -- a wvt-fed warm would schedule right
            # when m becomes ready and delay the projection)
            for _ in range(2):
                nc.tensor.matmul(
                    pw[:],
                    lhsT=hb_sb[:, 0, 0:2],
                    rhs=hb_sb[:, 0, 0:512],
                    start=True,
                    stop=True,
                )

            # projection: z[b, o] = sum_d m[d, b]*wvt[d, o] + bv[o]; the
            # rank-1 bias matmul starts each accumulation group (it
            # depends only on the tiny early loads)
            z_sb = small.tile([B, HID], BF16)
            for h, weng in ((0, nc.scalar), (1, nc.sync)):
                zp = psum.tile([B, 512], F32, name=f"z{h}", tag=f"z{h}")
                nc.tensor.matmul(
                    zp[:],
                    lhsT=ones2,
                    rhs=bvS_sb[:, 2 + h * 512 : 2 + (h + 1) * 512],
                    start=True,
                    stop=False,
                )
                nc.tensor.matmul(
                    zp[:],
                    lhsT=m_sb[:],
                    rhs=wvt_sb[:, h * 512 : (h + 1) * 512],
                    start=False,
                    stop=True,
                )
                # evac (m carried the 1/S, bias is in the PSUM): z0
                # through ACT, z1 through the DVE, in parallel; each
                # half's write leaves on its own HWDGE ring right after
                with nc.allow_low_precision(reason="bf16 partials, host sums f32"):
                    if h == 0:
                        nc.scalar.copy(z_sb[:, 0:512], zp[:])
                    else:
                        nc.vector.tensor_copy(z_sb[:, 512:1024], zp[:])
                weng.dma_start(
                    zout[:, h * 512 : (h + 1) * 512], z_sb[:, h * 512 : (h + 1) * 512]
                )
    nc.compile()
    return nc


def get_nc():
    global _compiled
    if _compiled is None:
        _compiled = _build()
    return _compiled


def make_in_maps(inputs):
    hb = np.asarray(inputs["hidden_states_b"], dtype=np.float32)
    Wv = np.asarray(inputs["Wv"], dtype=np.float32)
    bv = np.asarray(inputs["bv"], dtype=np.float32)
    bvS = np.zeros((N_CORES, 1, 2 + HID), dtype=NPBF16)
    bvS[:, 0, 0:2] = 1  # bias-matmul stationary ones
    bvS[0, 0, 2:] = bv.astype(NPBF16)
    maps = []
    for c in range(N_CORES):
        sl = hb[:, :, c * D_LOC : (c + 1) * D_LOC].astype(NPBF16)  # [B, S, 128]
        # hbt[d, b, s] = hb[b, s, c*128 + d]: one 4KB run per (d, b)
        t = sl.transpose(2, 0, 1)
        wt = Wv[:, c * D_LOC : (c + 1) * D_LOC].T.astype(NPBF16)  # [128 d, HID o]
        maps.append(
            {
                "hbt": np.ascontiguousarray(t),
                "wvt": np.ascontiguousarray(wt),
                "bvS": bvS[c],
            }
        )
    return maps


def combine(results):
    # unshard for contraction sharding: sum the 8 partials (bias was
    # folded into core 0's partial, 1/S scaling done on-device), then
    # broadcast the unique per-batch row over the sequence axis
    z = results[0]["zout"].astype(np.float32)
    for c in range(1, N_CORES):
        z += results[c]["zout"].astype(np.float32)
    return np.ascontiguousarray(np.broadcast_to(z[:, None, :], (B, S, HID)))


def kernel(**inputs) -> np.ndarray:
    nc = get_nc()
    res = run_bass_kernel_spmd(nc, make_in_maps(inputs), list(range(N_CORES)))
    return combine(res.results)


# revision 21
# speedup vs baseline: 1.0782x; 1.0110x over previous
"""Bass/Trainium2 kernel for nn_Differential_Attention_60825326846200.

Mathematical reduction of the reference:
  scores[b,h,i,j] = (sum_d q[b,h,i,d] - k[b,h,i,d]) / sqrt(DH) + mask[b,i]
is constant over the key index j, so the softmax over j is exactly the
uniform distribution (1/S) regardless of q, k, and the mask.  Hence
  ctx[b,h,i,:] = mean_j v[b,h,j,:]          (independent of i)
  out[b,i,:]   = (mean_j hidden_b[b,j,:]) @ Wv.T + bv   for every i.
The q/k projections and the attention mask cancel exactly, and the output
is rank-1 along the sequence axis: 2048 identical rows per batch.

ONE SPMD launch, contraction-sharded (no cross-core exchange needed):
core c owns HID columns d in [128c, 128c+128).

  Because the hidden dim (not the sequence) is sharded, each core's
  sequence reduction is COMPLETE for its slice: it reads
  hidden_b[:, :, d_c], reduces over all 2048 positions (partitions = the
  128 hidden columns, so m[d, b] lands already transposed for the
  projection lhsT), then contracts its 128 columns with its Wv slice ->
  z_c[b, o], a contraction-partial of the unique output row.  Core 0's
  bias input carries bv (others zeros), added via a rank-1 PE matmul
  into the same PSUM accumulation; m is pre-scaled by the exact 1/S so
  the PSUM holds final values and the evacuation is a plain copy.

  Host unshard = the standard gather for contraction sharding: sum the
  8 partials [2, 1024] and broadcast over the sequence axis (the output
  is rank-1: every row within a batch is the same vector).

  The inputs stream in bf16 (cast on the host while laying out the
  shards -- the 2e-2 tolerance is far above the ~3.6e-3 this lands at,
  and the f32 baseline already ran its matmuls in TF32-width float32r):
  1.26MB in / 4KB out per core.  Measured HW behavior baked in:
  - HWDGE rings cost ~21ns per descriptor with a ~131GB/s byte ceiling;
    a descriptor is one per-partition contiguous run, so the stream is
    THREE transfers only (each batch row = one 4KB run per partition,
    wvt whole = 2KB runs).  Column-splitting wvt (1KB runs) measured
    descriptor-limited at ~47GB/s.  SWDGE (gpsimd dma) is ~30-50GB/s,
    tiny loads only; partition-sliced DMAs use only half the SDMA
    engines.
  - The sequence reduction runs on TWO engines in parallel: DVE
    free-axis reduce_sum for batch 0, ACT activation(Copy, accum_out =
    per-partition running sum) for batch 1.  Both are ~1ns/element;
    neither alone can hide under the stream tail.  The ~1.3us
    ACT_TABLE_LOAD runs on the ACT engine concurrently with the scalar
    sequencer's DMA issues (it does NOT block the ring); a dummy early
    ACT op pins it to body start.
  - PSUM evac: z half 0 through ACT, half 1 through the DVE, in
    parallel; each half's 1KB-run write leaves on its own HWDGE ring.
  - f32r matmuls need a moving free dim >= 2; PSUM banks are 2KB per
    partition so z is two [2, 512] tiles.
  - Fixed costs dominate what remains: ~7us prolog (pre-body semaphore
    waits + instruction load) and ~2.9us of counted epilog per launch,
    which is why everything fits in ONE launch.
"""

import numpy as np
import ml_dtypes

import concourse.bacc as bacc
import concourse.mybir as mybir
import concourse.tile as tile
from concourse.bass_utils import run_bass_kernel_spmd

N_CORES = 8
B, S, HID = 2, 2048, 1024
D_LOC = HID // N_CORES  # 128 hidden columns owned per core
F32 = mybir.dt.float32
F32R = mybir.dt.float32r
BF16 = mybir.dt.bfloat16
NPBF16 = ml_dtypes.bfloat16

_compiled = None


def _new_nc():
    return bacc.Bacc(
        "TRN2",
        target_bir_lowering=False,
        debug=False,
        enable_asserts=False,
        num_devices=N_CORES,
    )


def _build():
    """Single launch: complete seq-reduction of this core's column slice,
    projection through its Wv rows, contraction-partial out.
    Inputs:
      "hbt" [128, B, S] bf16: hbt[d, b, s] = hb[b, s, 128*core + d]
      "wvt" [128, HID] bf16: wvt[d, o] = Wv[o, 128*core+d]
      "bvS" [1, 2+HID] bf16: cols 0:2 ones (bias-matmul stationary),
        cols 2: bv on core 0 / zeros elsewhere
    Output "zout" [B, HID] bf16: this core's contraction-partial of the
    unique output row (bias included on core 0, 1/S applied)."""
    nc = _new_nc()
    hbt = nc.dram_tensor("hbt", [128, B, S], BF16, kind="ExternalInput").ap()
    wvt = nc.dram_tensor("wvt", [128, HID], BF16, kind="ExternalInput").ap()
    bvS = nc.dram_tensor("bvS", [1, 2 + HID], BF16, kind="ExternalInput").ap()
    zout = nc.dram_tensor("zout", [B, HID], BF16, kind="ExternalOutput").ap()

    with tile.TileContext(nc) as tc:
        with (
            tc.tile_pool(name="big", bufs=1) as big,
            tc.tile_pool(name="small", bufs=1) as small,
            tc.tile_pool(name="psum", bufs=1, space="PSUM") as psum,
        ):
            hb_sb = big.tile([128, B, S], BF16)
            wvt_sb = big.tile([128, HID], BF16)
            bvS_sb = small.tile([1, 2 + HID], BF16)

            # three big transfers (descriptor-minimal); wvt last on the
            # sync ring -- it gates only the final projection, which also
            # needs m; tiny bias row on SWDGE off the HWDGE rings
            nc.gpsimd.dma_start(bvS_sb[:], bvS[:])
            nc.sync.dma_start(hb_sb[:, 0], hbt[:, 0])
            # batch 1 lands as two pieces so the ACT reduce starts ~2.3us
            # earlier; the slower qACT ring pays a mild descriptor
            # penalty (2KB runs) but carries nothing else
            nc.scalar.dma_start(hb_sb[:, 1, 0 : S // 2], hbt[:, 1, 0 : S // 2])
            nc.scalar.dma_start(hb_sb[:, 1, S // 2 :], hbt[:, 1, S // 2 :])
            nc.sync.dma_start(wvt_sb[:], wvt[:])

            ones2 = bvS_sb[0:1, 0:2]
            scratch = small.tile([128, 512], BF16)
            nc.vector.memset(scratch[:], 1.0)
            # PE clock-gate warmups while the DMAs stream; the dummy ACT
            # op pins the ~1.3us ACT_TABLE_LOAD to body start (it runs on
            # the ACT engine, concurrent with the scalar sequencer's DMA
            # issues -- it does not block the ring)
            pw = psum.tile([2, 512], F32, name="pwarm", tag="pwarm")
            for _ in range(6):
                nc.tensor.matmul(
                    pw[:], lhsT=scratch[:, 0:2], rhs=scratch[:], start=True, stop=True
                )
            act_warm = small.tile([1, 2], BF16)
            with nc.allow_low_precision(reason="dummy table-load trigger"):
                nc.scalar.copy(act_warm[:], scratch[0:1, 0:2])

            # seq reduction, one batch per engine in parallel: DVE
            # free-axis reduce for batch 0; ACT activation(Copy) with
            # accum_out (per-partition sum over the free axis) for batch
            # 1, its full-size out going to a scratch tile
            act_sink = big.tile([128, S], BF16)
            mtmp = small.tile([128, 1], F32)
            macc = small.tile([128, 2], BF16)
            m_sb = small.tile([128, B], BF16)
            with nc.allow_low_precision(reason="bf16 in, f32 accumulate"):
                nc.vector.reduce_sum(mtmp[:], hb_sb[:, 0], axis=mybir.AxisListType.X)
                # exact 1/S mean scaling folded into each engine's own
                # column of the projection stationary (no cross-engine
                # combine step)
                nc.vector.tensor_scalar_mul(m_sb[:, 0:1], mtmp[:], 1.0 / S)
                for half in range(2):
                    nc.scalar.activation(
                        act_sink[:, half * (S // 2) : (half + 1) * (S // 2)],
                        hb_sb[:, 1, half * (S // 2) : (half + 1) * (S // 2)],
                        mybir.ActivationFunctionType.Copy,
                        scale=1.0 / S,
                        accum_out=macc[:, half : half + 1],
                    )
                nc.vector.tensor_add(m_sb[:, 1:2], macc[:, 0:1], macc[:, 1:2])
            # keep the PE clock ungated between the warmups and the
            # projection (kept off the critical path: warms fed by the
            # first stream only -- a wvt-fed warm would schedule right
            # when m becomes ready and delay the projection)
            for _ in range(2):
                nc.tensor.matmul(
                    pw[:],
                    lhsT=hb_sb[:, 0, 0:2],
                    rhs=hb_sb[:, 0, 0:512],
                    start=True,
                    stop=True,
                )

            # projection: z[b, o] = sum_d m[d, b]*wvt[d, o] + bv[o]; the
            # rank-1 bias matmul starts each accumulation group (it
            # depends only on the tiny early loads)
            z_sb = small.tile([B, HID], BF16)
            for h, weng in ((0, nc.scalar), (1, nc.sync)):
                zp = psum.tile([B, 512], F32, name=f"z{h}", tag=f"z{h}")
                nc.tensor.matmul(
                    zp[:],
                    lhsT=ones2,
                    rhs=bvS_sb[:, 2 + h * 512 : 2 + (h + 1) * 512],
                    start=True,
                    stop=False,
                )
                nc.tensor.matmul(
                    zp[:],
                    lhsT=m_sb[:],
                    rhs=wvt_sb[:, h * 512 : (h + 1) * 512],
                    start=False,
                    stop=True,
                )
                # evac (m carried the 1/S, bias is in the PSUM): z0
                # through ACT, z1 through the DVE, in parallel; each
                # half's write leaves on its own HWDGE ring right after
                with nc.allow_low_precision(reason="bf16 partials, host sums f32"):
                    if h == 0:
                        nc.scalar.copy(z_sb[:, 0:512], zp[:])
                    else:
                        nc.vector.tensor_copy(z_sb[:, 512:1024], zp[:])
                weng.dma_start(
                    zout[:, h * 512 : (h + 1) * 512], z_sb[:, h * 512 : (h + 1) * 512]
                )
    nc.compile()
    return nc


def get_nc():
    global _compiled
    if _compiled is None:
        _compiled = _build()
    return _compiled


def make_in_maps(inputs):
    hb = np.asarray(inputs["hidden_states_b"], dtype=np.float32)
    Wv = np.asarray(inputs["Wv"], dtype=np.float32)
    bv = np.asarray(inputs["bv"], dtype=np.float32)
    bvS = np.zeros((N_CORES, 1, 2 + HID), dtype=NPBF16)
    bvS[:, 0, 0:2] = 1  # bias-matmul stationary ones
    bvS[0, 0, 2:] = bv.astype(NPBF16)
    maps = []
    for c in range(N_CORES):
        sl = hb[:, :, c * D_LOC : (c + 1) * D_LOC].astype(NPBF16)  # [B, S, 128]
        # hbt[d, b, s] = hb[b, s, c*128 + d]: one 4KB run per (d, b)
        t = sl.transpose(2, 0, 1)
        wt = Wv[:, c * D_LOC : (c + 1) * D_LOC].T.astype(NPBF16)  # [128 d, HID o]
        maps.append(
            {
                "hbt": np.ascontiguousarray(t),
                "wvt": np.ascontiguousarray(wt),
                "bvS": bvS[c],
            }
        )
    return maps


def combine(results):
    # unshard for contraction sharding: sum the 8 partials (bias was
    # folded into core 0's partial, 1/S scaling done on-device), then
    # broadcast the unique per-batch row over the sequence axis
    z = results[0]["zout"].astype(np.float32)
    for c in range(1, N_CORES):
        z += results[c]["zout"].astype(np.float32)
    return np.ascontiguousarray(np.broadcast_to(z[:, None, :], (B, S, HID)))


def kernel(**inputs) -> np.ndarray:
    nc = get_nc()
    res = run_bass_kernel_spmd(nc, make_in_maps(inputs), list(range(N_CORES)))
    return combine(res.results)
